# revision 3
# baseline (speedup 1.0000x reference)
"""GAT layer (gnn_message_passing) Trainium2 Bass kernel.

Math (per event b, N=6 nodes, D=64 features, H=1 head):
    x   = ((node @ W0.T + b0) @ W1.T + b1) @ W2.T + b2      # affine chain
        = node @ W_c + b_c            (W_c, b_c host-precomposed)
    es  = x @ a_src ; ed = x @ a_dst
    z_ij = lrelu(es_i + ed_j)  masked to -9e15 where adj==0
    P   = softmax_j(z) ; out_i = sum_j P_ij x_j

Device mapping (pure data parallel over events, 8 cores):
  - 128 events per sub-tile (events on SBUF partitions).
  - PE: 3 transposes [128,128] -> nT (features x events), then 3 matmuls
    lhsT=nT-pair, rhs=[blockdiag(W_c,W_c) | e-columns] produce x~ (no bias)
    AND es~/ed~ (no bias) directly event-major.
  - softmax rows always non-empty for this adj; mask applied as additive
    -9e15 BEFORE lrelu (exp -> 0 identically), row-max subtraction skipped
    (logits are O(10)); e-biases folded into the mask/fill constant.
  - x-bias b_c folds into aggregation: softmax rows sum to 1 so
    P @ (x~ + b_c) = P @ x~ + b_c  ->  b_c seeds the first FMA accumulator.
  - aggregation: one fused DVE scalar_tensor_tensor FMA per adjacency edge.
"""

import numpy as np

import concourse.bass as bass
import concourse.bacc as bacc
import concourse.tile as tile
import concourse.mybir as mybir
from concourse.masks import make_identity
from concourse.bass_utils import run_bass_kernel_spmd

F32 = mybir.dt.float32
ALU = mybir.AluOpType
ACTF = mybir.ActivationFunctionType

N_CORES = 8
B_FULL = 262144
NN = 6            # nodes per event
D = 64            # feature dim
TOK = NN * D      # 384 floats per event
EV_TILE = 128     # events per sub-tile (partition dim)
SUB_PER_BATCH = 4 # sub-tiles per DMA batch
BATCH_EV = EV_TILE * SUB_PER_BATCH  # 512 events per DMA


def _kept_rows(adj: np.ndarray):
    """[(i, [j,...]), ...] for rows of adj; adj is (6,6) 0/1."""
    rows = []
    for i in range(NN):
        js = [j for j in range(NN) if adj[i, j] == 1]
        rows.append((i, js))
    return rows


def _xcol(j: int) -> int:
    """Column of x~_j inside the per-event X tile [128, 396]."""
    return 64 * j


def build_program(b_shard: int, adj: np.ndarray):
    """Build + compile the per-core Bass program. adj values are baked into
    the instruction stream (which (i,j) FMAs exist), not into weights."""
    assert b_shard % BATCH_EV == 0
    nbatch = b_shard // BATCH_EV
    kept = _kept_rows(adj)
    for i, js in kept:
        assert js, "empty adjacency row needs the uniform-softmax path"

    nc = bacc.Bacc("TRN2", target_bir_lowering=False, debug=False)

    node_d = nc.dram_tensor("node", [b_shard, TOK], F32, kind="ExternalInput")
    rhs_d = nc.dram_tensor("rhsw", [128, 132], F32, kind="ExternalInput")
    bc_d = nc.dram_tensor("bc", [1, D], F32, kind="ExternalInput")
    fill_d = nc.dram_tensor("fillp", [1, NN * NN], F32, kind="ExternalInput")
    out_d = nc.dram_tensor("out", [b_shard, TOK], F32, kind="ExternalOutput")

    def ap4(t, off, dims):
        """Custom AP on tile t: partition dim kept, free dims replaced."""
        base = t[:]
        return bass.AP(tensor=base.tensor, offset=base.offset + off,
                       ap=[list(base.ap[0])] + [list(d) for d in dims])

    with tile.TileContext(nc) as tc:
        with (
            tc.tile_pool(name="singles", bufs=1) as singles,
            tc.tile_pool(name="loads", bufs=3) as loads,
            tc.tile_pool(name="nts", bufs=3) as ntsp,
            tc.tile_pool(name="xs", bufs=3) as xsp,
            tc.tile_pool(name="att", bufs=4) as att,
            tc.tile_pool(name="outs", bufs=3) as outs,
            tc.tile_pool(name="psA", bufs=2, space="PSUM") as psA,
            tc.tile_pool(name="psB", bufs=2, space="PSUM") as psB,
        ):
            ident = singles.tile([128, 128], F32)
            make_identity(nc, ident)
            rhs_s = singles.tile([128, 132], F32)
            nc.sync.dma_start(out=rhs_s, in_=rhs_d[:, :])
            bc_s = singles.tile([128, D], F32)
            nc.sync.dma_start(out=bc_s, in_=bc_d[0:1, :].partition_broadcast(128))
            fill_s = singles.tile([128, NN * NN], F32)
            nc.sync.dma_start(out=fill_s, in_=fill_d[0:1, :].partition_broadcast(128))

            for t in range(nbatch):
                ev0 = t * BATCH_EV
                nodeb = loads.tile([128, SUB_PER_BATCH, TOK], F32)
                nc.sync.dma_start(
                    out=nodeb,
                    in_=node_d[ev0:ev0 + BATCH_EV, :].rearrange(
                        "(s p) d -> p s d", p=128),
                )
                outb = outs.tile([128, SUB_PER_BATCH, TOK], F32)
                for s in range(SUB_PER_BATCH):
                    node_t = nodeb[:, s, :]
                    nT = psA.tile([128, TOK], F32)
                    for v in range(3):
                        nc.tensor.transpose(
                            nT[:, v * 128:(v + 1) * 128],
                            node_t[:, v * 128:(v + 1) * 128],
                            ident,
                        )
                    nTs = ntsp.tile([128, TOK], F32)
                    nc.scalar.copy(nTs, nT)

                    # X layout: x~_j at cols 64j (j=0..5); es_j at 384+2j,
                    # ed_j at 385+2j.
                    X = psB.tile([128, 396], F32)
                    for v in range(3):
                        nc.tensor.matmul(
                            X[:, v * 128:(v + 1) * 128],
                            lhsT=nTs[:, v * 128:(v + 1) * 128],
                            rhs=rhs_s[:, 0:128],
                            start=True, stop=True,
                        )
                        nc.tensor.matmul(
                            X[:, 384 + 4 * v:384 + 4 * v + 4],
                            lhsT=nTs[:, v * 128:(v + 1) * 128],
                            rhs=rhs_s[:, 128:132],
                            start=True, stop=True,
                        )
                    Xs = xsp.tile([128, 396], F32)
                    nc.scalar.copy(Xs, X)

                    # --- attention ---
                    # t1[i,j] = es_i + ed_j
                    es_ap = ap4(Xs, 384, [(2, 6), (0, 6)])
                    ed_ap = ap4(Xs, 385, [(0, 6), (2, 6)])
                    t1 = att.tile([128, NN * NN], F32)
                    t1_2d = ap4(t1, 0, [(6, 6), (1, 6)])
                    nc.vector.scalar_tensor_tensor(
                        out=t1_2d, in0=es_ap, scalar=0.0, in1=ed_ap,
                        op0=ALU.add, op1=ALU.add)
                    # z = t1 + (mask_fill + e-bias)
                    z = att.tile([128, NN * NN], F32)
                    nc.vector.scalar_tensor_tensor(
                        out=z[:], in0=t1[:], scalar=0.0, in1=fill_s[:],
                        op0=ALU.add, op1=ALU.add)
                    # lrelu: max(alpha*z, z)
                    lr = att.tile([128, NN * NN], F32)
                    nc.vector.scalar_tensor_tensor(
                        out=lr[:], in0=z[:], scalar=0.2, in1=z[:],
                        op0=ALU.mult, op1=ALU.max)
                    # n = exp(lr)  (ACT)
                    n_ = att.tile([128, NN * NN], F32)
                    nc.scalar.activation(n_[:], lr[:], ACTF.Exp)
                    # s_i = sum_j n_ij
                    s_ = att.tile([128, NN], F32)
                    n3 = ap4(n_, 0, [(6, 6), (1, 6)])
                    nc.vector.tensor_reduce(
                        out=s_[:], in_=n3, axis=mybir.AxisListType.X, op=ALU.add)
                    r_ = att.tile([128, NN], F32)
                    nc.vector.reciprocal(r_[:], s_[:])
                    # P = n * r_bcast
                    P_ = att.tile([128, NN * NN], F32)
                    r_b = ap4(r_, 0, [(1, 6), (0, 6)])
                    n2 = ap4(n_, 0, [(6, 6), (1, 6)])
                    P2 = ap4(P_, 0, [(6, 6), (1, 6)])
                    nc.vector.scalar_tensor_tensor(
                        out=P2, in0=n2, scalar=0.0, in1=r_b,
                        op0=ALU.add, op1=ALU.mult)
                    # aggregation: out_i = b_c + sum_j P_ij * x~_j
                    for i, js in kept:
                        out_i = outb[:, s, i * D:(i + 1) * D]
                        for idx, j in enumerate(js):
                            acc = bc_s[:, :] if idx == 0 else out_i
                            nc.vector.scalar_tensor_tensor(
                                out=out_i,
                                in0=Xs[:, _xcol(j):_xcol(j) + D],
                                scalar=P_[:, 6 * i + j:6 * i + j + 1],
                                in1=acc,
                                op0=ALU.mult, op1=ALU.add)
                nc.sync.dma_start(
                    out=out_d[ev0:ev0 + BATCH_EV, :].rearrange(
                        "(s p) d -> p s d", p=128),
                    in_=outb,
                )
    nc.compile()
    return nc


def host_consts(adj, W0, b0, W1, b1, W2, b2, a):
    """Host-side constant folding (float64 for the tiny compositions)."""
    W0d, W1d, W2d = (x.astype(np.float64) for x in (W0, W1, W2))
    b0d, b1d, b2d = (x.astype(np.float64) for x in (b0, b1, b2))
    W_c = (W2d @ W1d @ W0d).T                      # x~ = node @ W_c
    b_c = ((b0d @ W1d.T) + b1d) @ W2d.T + b2d
    a_src = a[0, :D].astype(np.float64)
    a_dst = a[0, D:].astype(np.float64)
    u_s = W_c @ a_src
    u_d = W_c @ a_dst
    c_sd = float(b_c @ a_src + b_c @ a_dst)

    rhs = np.zeros((128, 132), np.float32)
    for p in range(2):
        rhs[p * 64:(p + 1) * 64, p * 64:(p + 1) * 64] = W_c.astype(np.float32)
        rhs[p * 64:(p + 1) * 64, 128 + 2 * p] = u_s.astype(np.float32)
        rhs[p * 64:(p + 1) * 64, 129 + 2 * p] = u_d.astype(np.float32)
    bc = b_c.astype(np.float32).reshape(1, D)
    fillp = np.where(adj.reshape(-1) == 1, 0.0, -9e15).astype(np.float32)
    fillp = (fillp + np.float32(c_sd)).reshape(1, NN * NN).astype(np.float32)
    return rhs, bc, fillp


_prog_cache: dict = {}


def kernel(node, adj, W0, b0, W1, b1, W2, b2, a):
    node = np.ascontiguousarray(node, dtype=np.float32)
    adj = np.asarray(adj)
    B = node.shape[0]
    assert node.shape == (B, NN, D)
    b_shard = B // N_CORES

    key = (b_shard, adj.tobytes())
    if key not in _prog_cache:
        _prog_cache[key] = build_program(b_shard, adj)
    nc = _prog_cache[key]

    rhs, bc, fillp = host_consts(adj, W0, b0, W1, b1, W2, b2, a)
    shards = node.reshape(N_CORES, b_shard, TOK)
    in_maps = [
        {"node": shards[i], "rhsw": rhs, "bc": bc, "fillp": fillp}
        for i in range(N_CORES)
    ]
    res = run_bass_kernel_spmd(nc, in_maps, list(range(N_CORES)))
    out = np.stack([res.results[i]["out"] for i in range(N_CORES)])
    return out.reshape(B, NN, D)


# revision 13
# speedup vs baseline: 1.0909x; 1.0909x over previous
"""GAT layer (gnn_message_passing) Trainium2 Bass kernel.

Math (per event b, N=6 nodes, D=64 features, H=1 head):
    x   = ((node @ W0.T + b0) @ W1.T + b1) @ W2.T + b2      # affine chain
        = node @ W_c + b_c            (W_c, b_c host-precomposed)
    es  = x @ a_src ; ed = x @ a_dst
    z_ij = lrelu(es_i + ed_j)  masked to -9e15 where adj==0
    P   = softmax_j(z) ; out_i = sum_j P_ij x_j

Device mapping (pure data parallel over events, 8 cores):
  - 128 events per sub-tile (events on SBUF partitions).
  - PE: 3 transposes [128,128] -> nT (features x events), then 3 matmuls
    lhsT=nT-pair, rhs=[blockdiag(W_c,W_c) | e-columns] produce x~ (no bias)
    AND es~/ed~ (no bias) directly event-major.
  - softmax rows always non-empty for this adj; mask applied as additive
    -9e15 BEFORE lrelu (exp -> 0 identically), row-max subtraction skipped
    (logits are O(10)); e-biases folded into the mask/fill constant.
  - x-bias b_c folds into aggregation: softmax rows sum to 1 so
    P @ (x~ + b_c) = P @ x~ + b_c  ->  b_c seeds the first FMA accumulator.
  - aggregation: one fused DVE scalar_tensor_tensor FMA per adjacency edge.
"""

import numpy as np

import concourse.bass as bass
import concourse.bacc as bacc
import concourse.tile as tile
import concourse.mybir as mybir
from concourse.masks import make_identity
from concourse.bass_utils import run_bass_kernel_spmd

F32 = mybir.dt.float32
ALU = mybir.AluOpType
ACTF = mybir.ActivationFunctionType

N_CORES = 8
B_FULL = 262144
NN = 6            # nodes per event
D = 64            # feature dim
TOK = NN * D      # 384 floats per event
EV_TILE = 128     # events per sub-tile (partition dim)
SUB_PER_BATCH = 4 # sub-tiles per DMA batch
BATCH_EV = EV_TILE * SUB_PER_BATCH  # 512 events per DMA


def _kept_rows(adj: np.ndarray):
    """[(i, [j,...]), ...] for rows of adj; adj is (6,6) 0/1."""
    rows = []
    for i in range(NN):
        js = [j for j in range(NN) if adj[i, j] == 1]
        rows.append((i, js))
    return rows


def _xcol(j: int) -> int:
    """Column of x~_j inside the per-event X tile [128, 396]."""
    return 64 * j


AGG_GPS_ROWS = (0, 2)      # adjacency rows whose FMAs run on GPSIMD (t_t pairs)
GPS_RECIP = False          # reciprocal is DVE-only (no gpsimd lowering)
ACT_LRELU = True           # leaky-relu on ACT instead of DVE


def build_program(b_shard: int, adj: np.ndarray):
    """Build + compile the per-core Bass program. adj values are baked into
    the instruction stream (which (i,j) FMAs exist), not into weights."""
    assert b_shard % BATCH_EV == 0
    nbatch = b_shard // BATCH_EV
    kept = _kept_rows(adj)
    for i, js in kept:
        assert js, "empty adjacency row needs the uniform-softmax path"

    nc = bacc.Bacc("TRN2", target_bir_lowering=False, debug=False)

    node_d = nc.dram_tensor("node", [b_shard, TOK], F32, kind="ExternalInput")
    rhs_d = nc.dram_tensor("rhsw", [128, 132], F32, kind="ExternalInput")
    bc_d = nc.dram_tensor("bc", [1, D], F32, kind="ExternalInput")
    fill_d = nc.dram_tensor("fillp", [1, NN * NN], F32, kind="ExternalInput")
    out_d = nc.dram_tensor("out", [b_shard, TOK], F32, kind="ExternalOutput")

    def ap4(t, off, dims):
        """Custom AP on tile t: partition dim kept, free dims replaced."""
        base = t[:]
        return bass.AP(tensor=base.tensor, offset=base.offset + off,
                       ap=[list(base.ap[0])] + [list(d) for d in dims])

    with tile.TileContext(nc) as tc:
        with (
            tc.tile_pool(name="singles", bufs=1) as singles,
            tc.tile_pool(name="loads", bufs=3) as loads,
            tc.tile_pool(name="nts", bufs=3) as ntsp,
            tc.tile_pool(name="xs", bufs=3) as xsp,
            tc.tile_pool(name="att", bufs=4) as att,
            tc.tile_pool(name="outs", bufs=3) as outs,
            tc.tile_pool(name="psA", bufs=2, space="PSUM") as psA,
            tc.tile_pool(name="psB", bufs=2, space="PSUM") as psB,
        ):
            ident = singles.tile([128, 128], F32)
            make_identity(nc, ident)
            rhs_s = singles.tile([128, 132], F32)
            nc.sync.dma_start(out=rhs_s, in_=rhs_d[:, :])
            bc_s = singles.tile([128, D], F32)
            nc.sync.dma_start(out=bc_s, in_=bc_d[0:1, :].partition_broadcast(128))
            fill_s = singles.tile([128, NN * NN], F32)
            nc.sync.dma_start(out=fill_s, in_=fill_d[0:1, :].partition_broadcast(128))

            for t in range(nbatch):
                ev0 = t * BATCH_EV
                nodeb = loads.tile([128, SUB_PER_BATCH, TOK], F32)
                nc.sync.dma_start(
                    out=nodeb,
                    in_=node_d[ev0:ev0 + BATCH_EV, :].rearrange(
                        "(s p) d -> p s d", p=128),
                )
                outb = outs.tile([128, SUB_PER_BATCH, TOK], F32)
                for s in range(SUB_PER_BATCH):
                    node_t = nodeb[:, s, :]
                    nT = psA.tile([128, TOK], F32)
                    for v in range(3):
                        nc.tensor.transpose(
                            nT[:, v * 128:(v + 1) * 128],
                            node_t[:, v * 128:(v + 1) * 128],
                            ident,
                        )
                    nTs = ntsp.tile([128, TOK], F32)
                    nc.scalar.copy(nTs, nT)

                    # Merged matmul: per pair v one [128,132] output block:
                    # x_{2v} | x_{2v+1} | (es,ed for both j) — one stationary
                    # load per pair instead of two.
                    X = psB.tile([128, 396], F32)
                    for v in range(3):
                        nc.tensor.matmul(
                            X[:, v * 132:(v + 1) * 132],
                            lhsT=nTs[:, v * 128:(v + 1) * 128],
                            rhs=rhs_s[:, :],
                            start=True, stop=True,
                        )
                    # x~ cols: psum 132v + 64p + d -> Xs col 64j (incl dead x_5)
                    Xs = xsp.tile([128, TOK], F32)
                    nc.scalar.copy(
                        Xs[:], ap4(X, 0, [(132, 3), (1, 128)]))

                    # es_j at psum col 132v+128+2p; ed_j at +129 -> compact
                    es_s = att.tile([128, NN], F32)
                    nc.scalar.copy(es_s[:], ap4(X, 128, [(132, 3), (2, 2)]))
                    ed_s = att.tile([128, NN], F32)
                    nc.scalar.copy(ed_s[:], ap4(X, 129, [(132, 3), (2, 2)]))

                    # --- attention ---
                    # t1[i,j] = es_i + ed_j
                    t1 = att.tile([128, NN * NN], F32)
                    nc.vector.scalar_tensor_tensor(
                        out=ap4(t1, 0, [(6, 6), (1, 6)]),
                        in0=ap4(es_s, 0, [(1, 6), (0, 6)]),
                        scalar=0.0,
                        in1=ap4(ed_s, 0, [(0, 6), (1, 6)]),
                        op0=ALU.add, op1=ALU.add)
                    # z = t1 + (mask_fill + e-bias)
                    z = att.tile([128, NN * NN], F32)
                    nc.vector.scalar_tensor_tensor(
                        out=z[:], in0=t1[:], scalar=0.0, in1=fill_s[:],
                        op0=ALU.add, op1=ALU.add)
                    # lrelu: max(alpha*z, z), then n = exp(lr)
                    # (Prelu == parametric leaky-relu; unlike Lrelu it lives
                    # in the same ACT table as Exp/Copy -> no table thrash)
                    n_ = att.tile([128, NN * NN], F32)
                    if ACT_LRELU:
                        lr = att.tile([128, NN * NN], F32)
                        nc.scalar.activation(lr[:], z[:], ACTF.Prelu,
                                             alpha=0.2)
                        nc.scalar.activation(n_[:], lr[:], ACTF.Exp)
                    else:
                        lr = att.tile([128, NN * NN], F32)
                        nc.vector.scalar_tensor_tensor(
                            out=lr[:], in0=z[:], scalar=0.2, in1=z[:],
                            op0=ALU.mult, op1=ALU.max)
                        nc.scalar.activation(n_[:], lr[:], ACTF.Exp)
                    # s_i = sum_j n_ij
                    s_ = att.tile([128, NN], F32)
                    n3 = ap4(n_, 0, [(6, 6), (1, 6)])
                    nc.vector.tensor_reduce(
                        out=s_[:], in_=n3, axis=mybir.AxisListType.X, op=ALU.add)
                    r_ = att.tile([128, NN], F32)
                    (nc.gpsimd if GPS_RECIP else nc.vector).reciprocal(
                        r_[:], s_[:])
                    # P = n * r_bcast
                    P_ = att.tile([128, NN * NN], F32)
                    r_b = ap4(r_, 0, [(1, 6), (0, 6)])
                    n2 = ap4(n_, 0, [(6, 6), (1, 6)])
                    P2 = ap4(P_, 0, [(6, 6), (1, 6)])
                    nc.vector.scalar_tensor_tensor(
                        out=P2, in0=n2, scalar=0.0, in1=r_b,
                        op0=ALU.add, op1=ALU.mult)
                    # aggregation: out_i = b_c + sum_j P_ij * x~_j
                    # DVE rows: fused per-partition-scalar FMA (stt).
                    # GPS rows: walrus rejects TensorScalarPtr on Pool, so
                    # use tensor_tensor pairs (mult with P broadcast, add).
                    for i, js in kept:
                        out_i = outb[:, s, i * D:(i + 1) * D]
                        if i in AGG_GPS_ROWS:
                            tmp = att.tile([128, D], F32, tag="aggtmp")
                            for idx, j in enumerate(js):
                                p_b = ap4(P_, 6 * i + j, [(0, D)])
                                dst = out_i if idx == 0 else tmp
                                nc.gpsimd.tensor_tensor(
                                    out=dst,
                                    in0=Xs[:, _xcol(j):_xcol(j) + D],
                                    in1=p_b, op=ALU.mult)
                                nc.gpsimd.tensor_tensor(
                                    out=out_i, in0=dst if idx == 0 else tmp,
                                    in1=bc_s[:, :] if idx == 0 else out_i,
                                    op=ALU.add)
                        else:
                            for idx, j in enumerate(js):
                                acc = bc_s[:, :] if idx == 0 else out_i
                                nc.vector.scalar_tensor_tensor(
                                    out=out_i,
                                    in0=Xs[:, _xcol(j):_xcol(j) + D],
                                    scalar=P_[:, 6 * i + j:6 * i + j + 1],
                                    in1=acc,
                                    op0=ALU.mult, op1=ALU.add)
                nc.sync.dma_start(
                    out=out_d[ev0:ev0 + BATCH_EV, :].rearrange(
                        "(s p) d -> p s d", p=128),
                    in_=outb,
                )
    nc.compile()
    return nc


def host_consts(adj, W0, b0, W1, b1, W2, b2, a):
    """Host-side constant folding (float64 for the tiny compositions)."""
    W0d, W1d, W2d = (x.astype(np.float64) for x in (W0, W1, W2))
    b0d, b1d, b2d = (x.astype(np.float64) for x in (b0, b1, b2))
    W_c = (W2d @ W1d @ W0d).T                      # x~ = node @ W_c
    b_c = ((b0d @ W1d.T) + b1d) @ W2d.T + b2d
    a_src = a[0, :D].astype(np.float64)
    a_dst = a[0, D:].astype(np.float64)
    u_s = W_c @ a_src
    u_d = W_c @ a_dst
    c_sd = float(b_c @ a_src + b_c @ a_dst)

    rhs = np.zeros((128, 132), np.float32)
    for p in range(2):
        rhs[p * 64:(p + 1) * 64, p * 64:(p + 1) * 64] = W_c.astype(np.float32)
        rhs[p * 64:(p + 1) * 64, 128 + 2 * p] = u_s.astype(np.float32)
        rhs[p * 64:(p + 1) * 64, 129 + 2 * p] = u_d.astype(np.float32)
    bc = b_c.astype(np.float32).reshape(1, D)
    fillp = np.where(adj.reshape(-1) == 1, 0.0, -9e15).astype(np.float32)
    fillp = (fillp + np.float32(c_sd)).reshape(1, NN * NN).astype(np.float32)
    return rhs, bc, fillp


_prog_cache: dict = {}


def kernel(node, adj, W0, b0, W1, b1, W2, b2, a):
    node = np.ascontiguousarray(node, dtype=np.float32)
    adj = np.asarray(adj)
    B = node.shape[0]
    assert node.shape == (B, NN, D)
    b_shard = B // N_CORES

    key = (b_shard, adj.tobytes())
    if key not in _prog_cache:
        _prog_cache[key] = build_program(b_shard, adj)
    nc = _prog_cache[key]

    rhs, bc, fillp = host_consts(adj, W0, b0, W1, b1, W2, b2, a)
    shards = node.reshape(N_CORES, b_shard, TOK)
    in_maps = [
        {"node": shards[i], "rhsw": rhs, "bc": bc, "fillp": fillp}
        for i in range(N_CORES)
    ]
    res = run_bass_kernel_spmd(nc, in_maps, list(range(N_CORES)))
    out = np.stack([res.results[i]["out"] for i in range(N_CORES)])
    return out.reshape(B, NN, D)
